# revision 1
# baseline (speedup 1.0000x reference)
"""AttentionPooling Trainium2 kernel (8-core data-parallel).

Math: for each batch row b (B=2048, S=512, D=128):
    keys   = x @ Wk^T + bk + pos @ Wp^T + bp
    scores = (keys . q) * D**-0.5
    w      = softmax(scores)
    out    = sum_s w_s * (x_s @ Wv^T + bv)

Folding the fixed query into the projections collapses this to
    score[b,s] = x[b,s,:] . qk + pos[b,s,:] . qp   (+ const, which softmax drops)
        qk = Wk^T q * D**-0.5,  qp = Wp^T q * D**-0.5
    out[b]     = (sum_s e_s x_s) @ Wv^T / (sum_s e_s) + bv,  e = exp(score)
(sum w = 1 moves the value projection after the pooling; scores are O(0.1), so
exp needs no max-subtraction.)

Device layout per core (256 batches, data-parallel over 8 cores):
  tokens on partitions, 128-token groups; x tiles [128, 4b, 4g, 132] where
  cols 128:132 hold pos*qp (copied from a resident SBUF tile) so one fused
  DVE multiply-reduce per group yields the complete score. exp+sum on ACT,
  weighted token-sum on PE (contraction over the token partition dim),
  1/L + Wv projection + bias once per 128-batch block.
"""

import numpy as np

TOKEN_DIM = 128
SCALE = TOKEN_DIM ** -0.5
B, S, D = 2048, 512, 128
DC = D + 4                 # concat width: 128 x-cols + 4 pos-cols
NCORES = 8
BSH = B // NCORES          # 256 batches per core
G = S // 128               # 4 token groups of 128 per batch
BPI = 4                    # batches per inner iteration
NIT = BSH // BPI           # 64 iterations per core
BLK = 128                  # batches per output block (final projection granularity)
ITERS_PER_BLK = BLK // BPI
NBLK = BSH // BLK

_CACHE = {}


def _split_multi_waits(nc):
    """The walrus build here rejects instructions carrying more than one
    semaphore wait (limit varies by ISA struct; STT and Drain allow 1).
    Hoist extra waits onto same-engine NoOps placed just before the
    instruction — identical blocking semantics, trivial cost."""
    from concourse import mybir

    n = 0
    for f in nc.m.functions:
        for bb in f.blocks:
            new = []
            for inst in bb.instructions:
                si = inst.sync_info
                if si is not None and si.on_wait and len(si.on_wait) > 1:
                    waits = list(si.on_wait)
                    for w in waits[1:]:
                        n += 1
                        nop = mybir.InstNoOp(
                            name=f"T-wsplit-{n}", engine=inst.engine, ins=[], outs=[]
                        )
                        nop.sync_info = mybir.SyncInfo(on_wait=[w], on_update=[])
                        new.append(nop)
                    inst.sync_info = mybir.SyncInfo(
                        on_wait=[waits[0]], on_update=list(si.on_update or [])
                    )
                new.append(inst)
            bb.instructions = new
    return n


def build_program():
    """Build the per-core Bass program (SPMD across the 8 cores)."""
    import concourse.bass as bass
    import concourse.tile as tile
    from concourse import mybir

    f32 = mybir.dt.float32
    Exp = mybir.ActivationFunctionType.Exp
    Copy = mybir.ActivationFunctionType.Copy

    nc = bass.Bass("TRN2", target_bir_lowering=False, debug=False)
    x_d = nc.dram_tensor("x", [BSH, S, D], f32, kind="ExternalInput").ap()
    posq_d = nc.dram_tensor("posq", [128, BSH, G, 4], f32, kind="ExternalInput").ap()
    qkc_d = nc.dram_tensor("qkc", [128, DC], f32, kind="ExternalInput").ap()
    wvt_d = nc.dram_tensor("wvt", [D, D], f32, kind="ExternalInput").ap()
    bvb_d = nc.dram_tensor("bvb", [128, D], f32, kind="ExternalInput").ap()
    out_d = nc.dram_tensor("out", [BSH, D], f32, kind="ExternalOutput").ap()

    with tile.TileContext(nc) as tc:
        with (
            tc.tile_pool(name="consts", bufs=1) as consts,
            tc.tile_pool(name="posq", bufs=1) as posq_pool,
            tc.tile_pool(name="xin", bufs=4) as xin_pool,
            tc.tile_pool(name="scr", bufs=2) as scr_pool,
            tc.tile_pool(name="scores", bufs=3) as score_pool,
            tc.tile_pool(name="e", bufs=3) as e_pool,
            tc.tile_pool(name="tpsum", bufs=3, space="PSUM") as tpsum_pool,
            tc.tile_pool(name="Tblk", bufs=2) as Tblk_pool,
            tc.tile_pool(name="Lblk", bufs=2) as Lblk_pool,
            tc.tile_pool(name="epi_psum", bufs=2, space="PSUM") as epi_psum,
            tc.tile_pool(name="epi", bufs=2) as epi_pool,
        ):
            qkc_sb = consts.tile([128, DC], f32)
            nc.sync.dma_start(qkc_sb[:], qkc_d[:])
            wvt_sb = consts.tile([D, D], f32)
            nc.sync.dma_start(wvt_sb[:], wvt_d[:])
            bvb_sb = consts.tile([128, D], f32)
            nc.sync.dma_start(bvb_sb[:], bvb_d[:])
            ones_sb = consts.tile([128, 1], f32)
            nc.vector.memset(ones_sb[:], 1.0)

            # pos*qp stays resident in SBUF (2 MB, one line-rate DMA); per-iter
            # slices are copied into the concat columns of the x tile.
            posq_sb = posq_pool.tile([128, BSH, G, 4], f32)
            nc.sync.dma_start(posq_sb[:], posq_d[:])

            for blk in range(NBLK):
                Tblk = Tblk_pool.tile([128, BLK], f32)
                Lblk = Lblk_pool.tile([128, BLK], f32)
                for it in range(ITERS_PER_BLK):
                    i = blk * ITERS_PER_BLK + it
                    b0 = i * BPI
                    xin = xin_pool.tile([128, BPI, G, DC], f32)
                    nc.sync.dma_start(
                        xin[:, :, :, 0:D],
                        x_d[b0 : b0 + BPI].rearrange("b (g p) d -> p b g d", p=128),
                    )
                    nc.scalar.activation(
                        xin[:, :, :, D:DC], posq_sb[:, b0 : b0 + BPI, :, :], Copy
                    )
                    scores = score_pool.tile([128, BPI, G], f32)
                    e = e_pool.tile([128, BPI, G], f32)
                    scr = scr_pool.tile([128, DC], f32)
                    tpsum = tpsum_pool.tile([128, BPI], f32)
                    for bb in range(BPI):
                        for g in range(G):
                            nc.vector.scalar_tensor_tensor(
                                out=scr[:],
                                in0=xin[:, bb, g, :],
                                scalar=1.0,
                                in1=qkc_sb[:],
                                op0=mybir.AluOpType.mult,
                                op1=mybir.AluOpType.mult,
                                accum_out=scores[:, bb, g : g + 1],
                            )
                        nc.scalar.activation(
                            e[:, bb, :], scores[:, bb, :], Exp,
                            accum_out=Lblk[:, it * BPI + bb : it * BPI + bb + 1],
                        )
                        for g in range(G):
                            nc.tensor.matmul(
                                out=tpsum[:, bb : bb + 1],
                                lhsT=xin[:, bb, g, 0:D],
                                rhs=e[:, bb, g : g + 1],
                                start=(g == 0),
                                stop=(g == G - 1),
                            )
                    nc.scalar.activation(
                        Tblk[:, it * BPI : (it + 1) * BPI], tpsum[:], Copy
                    )
                # block epilogue: L per batch, 1/L, projection, bias, store
                Lp = epi_psum.tile([128, 1], f32, tag="Lp")
                nc.tensor.matmul(
                    out=Lp[:], lhsT=Lblk[:], rhs=ones_sb[:], start=True, stop=True
                )
                rcpL = epi_pool.tile([128, 1], f32, tag="rcpL")
                nc.vector.reciprocal(rcpL[:], Lp[:])
                proj = epi_psum.tile([128, D], f32, tag="proj")
                nc.tensor.matmul(
                    out=proj[:], lhsT=Tblk[:], rhs=wvt_sb[:], start=True, stop=True
                )
                scaled = epi_pool.tile([128, D], f32, tag="scaled")
                nc.scalar.activation(scaled[:], proj[:], Copy, scale=rcpL[:])
                out_sb = epi_pool.tile([128, D], f32, tag="out_sb")
                nc.vector.tensor_add(out_sb[:], scaled[:], bvb_sb[:])
                nc.sync.dma_start(out_d[blk * BLK : (blk + 1) * BLK, :], out_sb[:])

    _split_multi_waits(nc)
    return nc


def prepare_inputs(input_features, positions, mask, query, Wk, bk, Wv, bv, Wp, bp):
    """Host-side prep: shard along batch, replicate/fold the small weights."""
    q = np.asarray(query, np.float32)[0]
    qk = (q @ np.asarray(Wk, np.float32)) * SCALE           # [D]
    qp = (q @ np.asarray(Wp, np.float32)) * SCALE           # [4]
    # concat multiplier: qk over the x columns, 1.0 over the pos columns
    qkc = np.concatenate([qk, np.ones(4, np.float32)]).astype(np.float32)
    qkc = np.ascontiguousarray(np.broadcast_to(qkc[None, :], (128, DC)))
    wvt = np.ascontiguousarray(np.asarray(Wv, np.float32).T)
    bvb = np.ascontiguousarray(
        np.broadcast_to(np.asarray(bv, np.float32)[None, :], (128, D))
    )

    # pos repack: [B, S, 4] -> [128(p), B, G, 4] with qp folded in; masked
    # tokens get a -1e30 term so their softmax weight underflows to exactly 0.
    pos = np.asarray(positions, np.float32).reshape(B, G, 128, 4)
    posq = pos.transpose(2, 0, 1, 3) * qp[None, None, None, :]
    m = np.asarray(mask, bool)
    if not m.all():
        mb = m.reshape(B, G, 128).transpose(2, 0, 1)        # [p, B, G]
        posq = posq.copy()
        posq[..., 0] = np.where(mb, posq[..., 0], np.float32(-1e30))
    posq = np.ascontiguousarray(posq, np.float32)

    x = np.ascontiguousarray(np.asarray(input_features, np.float32))
    in_maps = []
    for c in range(NCORES):
        in_maps.append(
            {
                "x": x[c * BSH : (c + 1) * BSH],
                "posq": np.ascontiguousarray(posq[:, c * BSH : (c + 1) * BSH]),
                "qkc": qkc,
                "wvt": wvt,
                "bvb": bvb,
            }
        )
    return in_maps


def kernel(input_features, positions, mask, query, Wk, bk, Wv, bv, Wp, bp):
    from concourse.bass_utils import run_bass_kernel_spmd

    if "nc" not in _CACHE:
        _CACHE["nc"] = build_program()
    nc = _CACHE["nc"]
    in_maps = prepare_inputs(
        input_features, positions, mask, query, Wk, bk, Wv, bv, Wp, bp
    )
    res = run_bass_kernel_spmd(nc, in_maps, list(range(NCORES)))
    return np.concatenate([res.results[c]["out"] for c in range(NCORES)], axis=0)



# revision 2
# speedup vs baseline: 1.1968x; 1.1968x over previous
"""AttentionPooling Trainium2 kernel (8-core data-parallel), v2.

Math per batch row b (B=2048, S=512, D=128):
    keys   = x @ Wk^T + bk + pos @ Wp^T + bp
    scores = (keys . q) * D**-0.5
    w      = softmax(scores)
    out    = sum_s w_s * (x_s @ Wv^T + bv)

Folding the fixed query into the projections collapses this to
    score[b,s] = x[b,s,:] . qk + ps[b,s]        (qk = Wk^T q * SCALE)
    out[b]     = (sum_s e_s x_s) @ Wv^T / (sum_s e_s) + bv,   e = exp(score)
where ps = pos . qp (+ mask fold) is tiny and precomputed host-side
(same O(B*S*4) folding the v1 kernel did).

v2 device layout (per core, BSH=256 batches):
  Token scramble s = 4p + t: partition p holds 4 consecutive tokens of a
  batch, so the x DMA moves 2 KB contiguous DRAM runs per partition
  (v1's layout had 512 B runs, which capped the 16 DMA queues at
  ~280 GB/s aggregate; 2 KB runs reach line rate).

  Engine split, per 8-batch granule [128p, 8b, 4t, 128d]:
    - DMA      : f32 x granule in (2 KB packets)
    - ACT/Pool : f32 -> bf16 convert (granules round-robined 20/12 so
                 neither engine exceeds ~80 us)
    - DVE      : 32 scalar_tensor_tensor on bf16 (4x perf mode),
                 accum_out -> f32 scores; + ps add; + L-partial reduce
    - ACT      : exp (f32 -> bf16 e)
    - PE       : 4 bf16 matmuls per batch (stationary x tile, moving e
                 column) accumulating T[:,b] in PSUM. bf16 stationary
                 loads are 4x faster than v1's fp32 ones.
  Per 128-batch block: L via ones-matmul over DVE-reduced partials,
  1/L on DVE, T -> bf16, Wv^T projection (bf16 matmul), per-partition
  1/L scale on ACT, bias add, store.
"""

import numpy as np

TOKEN_DIM = 128
SCALE = TOKEN_DIM ** -0.5
B, S, D = 2048, 512, 128
NCORES = 8
BSH = B // NCORES          # 256 batches per core
TPB = 4                    # tokens per partition per batch (s = 4p + t)
GR = 8                     # batches per DMA granule
NGR = BSH // GR            # 32 granules per core
BLK = 128                  # batches per output block
GPB = BLK // GR            # granules per block (16)
NBLK = BSH // BLK          # 2

# Convert-engine assignment per granule in a block: 'P' = GpSimd, 'A' = ACT.
# 12/32 on Pool, 20/32 on ACT keeps both near ~70 us/core.
_CONV = ['P' if g % 8 in (1, 3, 5) else 'A' for g in range(NGR)]

_CACHE = {}


def _split_multi_waits(nc):
    """The walrus build here rejects instructions carrying more than one
    semaphore wait (limit varies by ISA struct; STT and Drain allow 1).
    Hoist extra waits onto same-engine NoOps placed just before the
    instruction — identical blocking semantics, trivial cost."""
    from concourse import mybir

    n = 0
    for f in nc.m.functions:
        for bb in f.blocks:
            new = []
            for inst in bb.instructions:
                si = inst.sync_info
                if si is not None and si.on_wait and len(si.on_wait) > 1:
                    waits = list(si.on_wait)
                    for w in waits[1:]:
                        n += 1
                        nop = mybir.InstNoOp(
                            name=f"T-wsplit-{n}", engine=inst.engine, ins=[], outs=[]
                        )
                        nop.sync_info = mybir.SyncInfo(on_wait=[w], on_update=[])
                        new.append(nop)
                    inst.sync_info = mybir.SyncInfo(
                        on_wait=[waits[0]], on_update=list(si.on_update or [])
                    )
                new.append(inst)
            bb.instructions = new
    return n


def build_program():
    """Build the per-core Bass program (SPMD across the 8 cores)."""
    import concourse.bass as bass
    import concourse.tile as tile
    from concourse import mybir

    f32 = mybir.dt.float32
    bf16 = mybir.dt.bfloat16
    Exp = mybir.ActivationFunctionType.Exp
    Copy = mybir.ActivationFunctionType.Copy
    Add = mybir.AluOpType.add
    Mult = mybir.AluOpType.mult

    nc = bass.Bass("TRN2", target_bir_lowering=False, debug=False)
    x_d = nc.dram_tensor("x", [BSH, S, D], f32, kind="ExternalInput").ap()
    ps_d = nc.dram_tensor("ps", [128, BSH, TPB], f32, kind="ExternalInput").ap()
    qkb_d = nc.dram_tensor("qkb", [128, D], bf16, kind="ExternalInput").ap()
    wvt_d = nc.dram_tensor("wvt", [D, D], bf16, kind="ExternalInput").ap()
    bvb_d = nc.dram_tensor("bvb", [128, D], f32, kind="ExternalInput").ap()
    out_d = nc.dram_tensor("out", [BSH, D], f32, kind="ExternalOutput").ap()

    with tile.TileContext(nc) as tc:
        with (
            tc.tile_pool(name="consts", bufs=1) as consts,
            tc.tile_pool(name="xf", bufs=4) as xf_pool,
            tc.tile_pool(name="xb", bufs=4) as xb_pool,
            tc.tile_pool(name="scr", bufs=2) as scr_pool,
            tc.tile_pool(name="sc", bufs=2) as sc_pool,
            tc.tile_pool(name="sce", bufs=2) as sce_pool,
            tc.tile_pool(name="e", bufs=3) as e_pool,
            tc.tile_pool(name="P", bufs=2) as P_pool,
            tc.tile_pool(name="tpsum", bufs=2, space="PSUM") as tpsum_pool,
            tc.tile_pool(name="epi_psum", bufs=2, space="PSUM") as epi_psum,
            tc.tile_pool(name="epi", bufs=4) as epi_pool,
        ):
            qkb_sb = consts.tile([128, D], bf16)
            nc.sync.dma_start(qkb_sb[:], qkb_d[:])
            wvt_sb = consts.tile([D, D], bf16)
            nc.sync.dma_start(wvt_sb[:], wvt_d[:])
            bvb_sb = consts.tile([128, D], f32)
            nc.sync.dma_start(bvb_sb[:], bvb_d[:])
            ps_sb = consts.tile([128, BSH, TPB], f32)
            nc.sync.dma_start(ps_sb[:], ps_d[:])
            ones_sb = consts.tile([128, 1], f32)
            nc.vector.memset(ones_sb[:], 1.0)

            for blk in range(NBLK):
                Tpsum = tpsum_pool.tile([128, BLK], f32)
                P_blk = P_pool.tile([128, BLK], f32)
                for gg in range(GPB):
                    g = blk * GPB + gg
                    b0 = g * GR
                    xf = xf_pool.tile([128, GR, TPB, D], f32)
                    nc.sync.dma_start(
                        xf[:],
                        x_d[b0 : b0 + GR].rearrange("b (p t) d -> p b t d", t=TPB),
                    )
                    xb = xb_pool.tile([128, GR, TPB, D], bf16)
                    if _CONV[g] == 'A':
                        nc.scalar.activation(xb[:], xf[:], Copy)
                    else:
                        nc.gpsimd.tensor_copy(xb[:], xf[:])

                    sc = sc_pool.tile([128, GR, TPB], f32)
                    for j in range(GR):
                        for t in range(TPB):
                            scr = scr_pool.tile([128, D], bf16)
                            nc.vector.scalar_tensor_tensor(
                                out=scr[:],
                                in0=xb[:, j, t, :],
                                scalar=1.0,
                                in1=qkb_sb[:],
                                op0=Mult,
                                op1=Mult,
                                accum_out=sc[:, j, t : t + 1],
                            )
                    sce = sce_pool.tile([128, GR, TPB], f32)
                    nc.vector.tensor_tensor(
                        out=sce[:], in0=sc[:], in1=ps_sb[:, b0 : b0 + GR, :], op=Add
                    )
                    e = e_pool.tile([128, GR, TPB], bf16)
                    nc.scalar.activation(e[:], sce[:], Exp)
                    nc.vector.tensor_reduce(
                        out=P_blk[:, gg * GR : (gg + 1) * GR],
                        in_=e[:],
                        axis=mybir.AxisListType.X,
                        op=Add,
                    )
                    for j in range(GR):
                        bcol = gg * GR + j
                        for t in range(TPB):
                            nc.tensor.matmul(
                                out=Tpsum[:, bcol : bcol + 1],
                                lhsT=xb[:, j, t, :],
                                rhs=e[:, j, t : t + 1],
                                start=(t == 0),
                                stop=(t == TPB - 1),
                            )
                # block epilogue
                Lp = epi_psum.tile([128, 1], f32, tag="Lp")
                nc.tensor.matmul(
                    out=Lp[:], lhsT=P_blk[:], rhs=ones_sb[:], start=True, stop=True
                )
                rcpL = epi_pool.tile([128, 1], f32, tag="rcpL")
                nc.vector.reciprocal(rcpL[:], Lp[:])
                Tsb = epi_pool.tile([128, BLK], bf16, tag="Tsb")
                nc.scalar.activation(Tsb[:], Tpsum[:], Copy)
                proj = epi_psum.tile([128, D], f32, tag="proj")
                nc.tensor.matmul(
                    out=proj[:], lhsT=Tsb[:], rhs=wvt_sb[:], start=True, stop=True
                )
                scaled = epi_pool.tile([128, D], f32, tag="scaled")
                nc.scalar.activation(scaled[:], proj[:], Copy, scale=rcpL[:])
                out_sb = epi_pool.tile([128, D], f32, tag="out_sb")
                nc.vector.tensor_tensor(
                    out=out_sb[:], in0=scaled[:], in1=bvb_sb[:], op=Add
                )
                nc.sync.dma_start(out_d[blk * BLK : (blk + 1) * BLK, :], out_sb[:])

    _split_multi_waits(nc)
    return nc


def prepare_inputs(input_features, positions, mask, query, Wk, bk, Wv, bv, Wp, bp):
    """Host-side prep: shard along batch, replicate/fold the small weights."""
    import ml_dtypes

    q = np.asarray(query, np.float32)[0]
    qk = (q @ np.asarray(Wk, np.float32)) * SCALE           # [D]
    qp = (q @ np.asarray(Wp, np.float32)) * SCALE           # [4]
    qkb = np.ascontiguousarray(
        np.broadcast_to(qk[None, :].astype(ml_dtypes.bfloat16), (128, D))
    )
    wvt = np.ascontiguousarray(np.asarray(Wv, np.float32).T.astype(ml_dtypes.bfloat16))
    bvb = np.ascontiguousarray(
        np.broadcast_to(np.asarray(bv, np.float32)[None, :], (128, D))
    )

    # ps[b, s] = pos . qp with masked tokens forced to -1e30 so their
    # softmax weight underflows to exactly 0. Packed as [128 p, B, 4 t]
    # matching the device's s = 4p + t token scramble.
    ps = np.asarray(positions, np.float32) @ qp              # [B, S]
    m = np.asarray(mask, bool)
    if not m.all():
        ps = np.where(m, ps, np.float32(-1e30))
    ps = np.ascontiguousarray(
        ps.reshape(B, 128, TPB).transpose(1, 0, 2), np.float32
    )

    x = np.asarray(input_features, np.float32)
    in_maps = []
    for c in range(NCORES):
        in_maps.append(
            {
                "x": x[c * BSH : (c + 1) * BSH],
                "ps": np.ascontiguousarray(ps[:, c * BSH : (c + 1) * BSH]),
                "qkb": qkb,
                "wvt": wvt,
                "bvb": bvb,
            }
        )
    return in_maps


def kernel(input_features, positions, mask, query, Wk, bk, Wv, bv, Wp, bp):
    from concourse.bass_utils import run_bass_kernel_spmd

    if "nc" not in _CACHE:
        _CACHE["nc"] = build_program()
    nc = _CACHE["nc"]
    in_maps = prepare_inputs(
        input_features, positions, mask, query, Wk, bk, Wv, bv, Wp, bp
    )
    res = run_bass_kernel_spmd(nc, in_maps, list(range(NCORES)))
    return np.concatenate([res.results[c]["out"] for c in range(NCORES)], axis=0)


# revision 3
# speedup vs baseline: 1.7610x; 1.4714x over previous
"""AttentionPooling Trainium2 kernel (8-core data-parallel), v3.

Math per batch row b (B=2048, S=512, D=128):
    score[b,s] = x[b,s,:] . qk + ps[b,s]        (qk = Wk^T q * SCALE)
    out[b]     = (sum_s e_s x_s) @ Wv^T / (sum_s e_s) + bv,   e = exp(score)
ps = pos . qp (+ mask fold to -1e30) is tiny and precomputed host-side
(same O(B*S*4) folding the v1 kernel did).

Device layout (per core, BSH=256 batches): token scramble s = 4p + t, so
partition p holds 4 consecutive tokens of a batch and the x DMA moves
2 KB contiguous DRAM runs (16 queues hit line rate ~360 GB/s; the DMA
floor ~188 us/core is the roofline for this memory-regime problem).

Engine split per 8-batch granule [128p, 8b, 4t, 128d], balanced so DVE /
ACT / DMA all land ~185 us (measured per-op rates via microbench):
  DMA   x granule f32 in (2 KB packets)
  ACT   f32 -> bf16 convert (28 of 32 granules, ~4.6 us each) + exp
  GpSimd  the other 4 converts (~14 us each) + ps-add + bias adds
  DVE   scores: bf16 TT-mult against broadcast qk (no STT accumulator —
        its ~460 ns/instr fixed cost killed v2), 4 strided bf16 halving
        folds 128->8, segmented tensor_reduce 8->1 into f32 scores;
        plus L-partial reduce over t.
  PE    4 bf16 matmuls per batch (stationary x tile, moving e column)
        accumulating T[:,b] PSUM columns; per-block: ones-matmul for L,
        bf16 Wv^T projection, ACT 1/L per-partition scale, bias, store.
"""

import numpy as np

TOKEN_DIM = 128
SCALE = TOKEN_DIM ** -0.5
B, S, D = 2048, 512, 128
NCORES = 8
BSH = B // NCORES          # 256 batches per core
TPB = 4                    # tokens per partition per batch (s = 4p + t)
GR = 8                     # batches per granule
NGR = BSH // GR            # 32 granules per core
BLK = 128                  # batches per output block
GPB = BLK // GR            # granules per block (16)
NBLK = BSH // BLK          # 2
SEG = GR * TPB             # 32 score segments per granule

# Convert-engine per granule: 4 of 32 on GpSimd, rest on ACT.
_CONV = ['P' if g % 8 == 5 else 'A' for g in range(NGR)]

_CACHE = {}


def _split_multi_waits(nc):
    """The walrus build here rejects instructions carrying more than one
    semaphore wait (limit varies by ISA struct; STT and Drain allow 1).
    Hoist extra waits onto same-engine NoOps placed just before the
    instruction — identical blocking semantics, trivial cost."""
    from concourse import mybir

    n = 0
    for f in nc.m.functions:
        for bb in f.blocks:
            new = []
            for inst in bb.instructions:
                si = inst.sync_info
                if si is not None and si.on_wait and len(si.on_wait) > 1:
                    waits = list(si.on_wait)
                    for w in waits[1:]:
                        n += 1
                        nop = mybir.InstNoOp(
                            name=f"T-wsplit-{n}", engine=inst.engine, ins=[], outs=[]
                        )
                        nop.sync_info = mybir.SyncInfo(on_wait=[w], on_update=[])
                        new.append(nop)
                    inst.sync_info = mybir.SyncInfo(
                        on_wait=[waits[0]], on_update=list(si.on_update or [])
                    )
                new.append(inst)
            bb.instructions = new
    return n


def build_program():
    """Build the per-core Bass program (SPMD across the 8 cores)."""
    import concourse.bass as bass
    import concourse.tile as tile
    from concourse import mybir

    f32 = mybir.dt.float32
    bf16 = mybir.dt.bfloat16
    Exp = mybir.ActivationFunctionType.Exp
    Copy = mybir.ActivationFunctionType.Copy
    Add = mybir.AluOpType.add
    Mult = mybir.AluOpType.mult
    X = mybir.AxisListType.X

    nc = bass.Bass("TRN2", target_bir_lowering=False, debug=False)
    x_d = nc.dram_tensor("x", [BSH, S, D], f32, kind="ExternalInput").ap()
    ps_d = nc.dram_tensor("ps", [128, BSH, TPB], f32, kind="ExternalInput").ap()
    qkr_d = nc.dram_tensor("qkr", [128, TPB * D], bf16, kind="ExternalInput").ap()
    wvt_d = nc.dram_tensor("wvt", [D, D], bf16, kind="ExternalInput").ap()
    bvb_d = nc.dram_tensor("bvb", [128, D], f32, kind="ExternalInput").ap()
    out_d = nc.dram_tensor("out", [BSH, D], f32, kind="ExternalOutput").ap()

    with tile.TileContext(nc) as tc:
        with (
            tc.tile_pool(name="consts", bufs=1) as consts,
            tc.tile_pool(name="xf", bufs=4) as xf_pool,
            tc.tile_pool(name="xb", bufs=4) as xb_pool,
            tc.tile_pool(name="prod", bufs=2) as prod_pool,
            tc.tile_pool(name="f1", bufs=2) as f1_pool,
            tc.tile_pool(name="f2", bufs=2) as f2_pool,
            tc.tile_pool(name="f3", bufs=2) as f3_pool,
            tc.tile_pool(name="f4", bufs=2) as f4_pool,
            tc.tile_pool(name="sc", bufs=2) as sc_pool,
            tc.tile_pool(name="sce", bufs=2) as sce_pool,
            tc.tile_pool(name="e", bufs=3) as e_pool,
            tc.tile_pool(name="P", bufs=2) as P_pool,
            tc.tile_pool(name="tpsum", bufs=2, space="PSUM") as tpsum_pool,
            tc.tile_pool(name="epi_psum", bufs=2, space="PSUM") as epi_psum,
            tc.tile_pool(name="epi", bufs=4) as epi_pool,
        ):
            qkr_sb = consts.tile([128, TPB * D], bf16)
            nc.sync.dma_start(qkr_sb[:], qkr_d[:])
            wvt_sb = consts.tile([D, D], bf16)
            nc.sync.dma_start(wvt_sb[:], wvt_d[:])
            bvb_sb = consts.tile([128, D], f32)
            nc.sync.dma_start(bvb_sb[:], bvb_d[:])
            ps_sb = consts.tile([128, BSH, TPB], f32)
            nc.sync.dma_start(ps_sb[:], ps_d[:])
            ones_sb = consts.tile([128, 1], f32)
            nc.vector.memset(ones_sb[:], 1.0)

            qk_b = qkr_sb[:].rearrange("p (o f) -> p o f", o=1).broadcast_to(
                [128, GR, TPB * D]
            )

            for blk in range(NBLK):
                Tpsum = tpsum_pool.tile([128, BLK], f32)
                P_blk = P_pool.tile([128, BLK], f32)
                for gg in range(GPB):
                    g = blk * GPB + gg
                    b0 = g * GR
                    xf = xf_pool.tile([128, GR, TPB, D], f32)
                    nc.sync.dma_start(
                        xf[:],
                        x_d[b0 : b0 + GR].rearrange("b (p t) d -> p b t d", t=TPB),
                    )
                    xb = xb_pool.tile([128, GR, TPB, D], bf16)
                    if _CONV[g] == 'A':
                        nc.scalar.activation(xb[:], xf[:], Copy)
                    else:
                        nc.gpsimd.tensor_copy(xb[:], xf[:])

                    # scores: prod = xb * qk (bf16), fold 128 -> 8, reduce -> f32
                    prod = prod_pool.tile([128, SEG, D], bf16)
                    nc.vector.tensor_tensor(
                        out=prod[:].rearrange("p s d -> p (s d)").rearrange(
                            "p (b f) -> p b f", b=GR
                        ),
                        in0=xb[:].rearrange("p b t d -> p b (t d)"),
                        in1=qk_b,
                        op=Mult,
                    )
                    f1 = f1_pool.tile([128, SEG, 64], bf16)
                    nc.vector.tensor_tensor(
                        out=f1[:], in0=prod[:, :, 0:64], in1=prod[:, :, 64:128], op=Add
                    )
                    f2 = f2_pool.tile([128, SEG, 32], bf16)
                    nc.vector.tensor_tensor(
                        out=f2[:], in0=f1[:, :, 0:32], in1=f1[:, :, 32:64], op=Add
                    )
                    f3 = f3_pool.tile([128, SEG, 16], bf16)
                    nc.vector.tensor_tensor(
                        out=f3[:], in0=f2[:, :, 0:16], in1=f2[:, :, 16:32], op=Add
                    )
                    f4 = f4_pool.tile([128, SEG, 8], bf16)
                    nc.vector.tensor_tensor(
                        out=f4[:], in0=f3[:, :, 0:8], in1=f3[:, :, 8:16], op=Add
                    )
                    sc = sc_pool.tile([128, GR, TPB], f32)
                    nc.vector.tensor_reduce(
                        out=sc[:].rearrange("p b t -> p (b t)"), in_=f4[:],
                        axis=X, op=Add,
                    )
                    sce = sce_pool.tile([128, GR, TPB], f32)
                    nc.gpsimd.tensor_tensor(
                        out=sce[:], in0=sc[:], in1=ps_sb[:, b0 : b0 + GR, :], op=Add
                    )
                    e = e_pool.tile([128, GR, TPB], bf16)
                    nc.scalar.activation(e[:], sce[:], Exp)
                    nc.vector.tensor_reduce(
                        out=P_blk[:, gg * GR : (gg + 1) * GR], in_=e[:], axis=X, op=Add
                    )
                    for j in range(GR):
                        bcol = gg * GR + j
                        for t in range(TPB):
                            nc.tensor.matmul(
                                out=Tpsum[:, bcol : bcol + 1],
                                lhsT=xb[:, j, t, :],
                                rhs=e[:, j, t : t + 1],
                                start=(t == 0),
                                stop=(t == TPB - 1),
                            )
                # block epilogue
                Lp = epi_psum.tile([128, 1], f32, tag="Lp")
                nc.tensor.matmul(
                    out=Lp[:], lhsT=P_blk[:], rhs=ones_sb[:], start=True, stop=True
                )
                rcpL = epi_pool.tile([128, 1], f32, tag="rcpL")
                nc.vector.reciprocal(rcpL[:], Lp[:])
                Tsb = epi_pool.tile([128, BLK], bf16, tag="Tsb")
                nc.scalar.activation(Tsb[:], Tpsum[:], Copy)
                proj = epi_psum.tile([128, D], f32, tag="proj")
                nc.tensor.matmul(
                    out=proj[:], lhsT=Tsb[:], rhs=wvt_sb[:], start=True, stop=True
                )
                scaled = epi_pool.tile([128, D], f32, tag="scaled")
                nc.scalar.activation(scaled[:], proj[:], Copy, scale=rcpL[:])
                out_sb = epi_pool.tile([128, D], f32, tag="out_sb")
                nc.gpsimd.tensor_tensor(
                    out=out_sb[:], in0=scaled[:], in1=bvb_sb[:], op=Add
                )
                nc.sync.dma_start(out_d[blk * BLK : (blk + 1) * BLK, :], out_sb[:])

    _split_multi_waits(nc)
    return nc


def prepare_inputs(input_features, positions, mask, query, Wk, bk, Wv, bv, Wp, bp):
    """Host-side prep: shard along batch, replicate/fold the small weights."""
    import ml_dtypes

    q = np.asarray(query, np.float32)[0]
    qk = (q @ np.asarray(Wk, np.float32)) * SCALE           # [D]
    qp = (q @ np.asarray(Wp, np.float32)) * SCALE           # [4]
    qkr = np.ascontiguousarray(
        np.broadcast_to(
            np.tile(qk, TPB)[None, :].astype(ml_dtypes.bfloat16), (128, TPB * D)
        )
    )
    wvt = np.ascontiguousarray(np.asarray(Wv, np.float32).T.astype(ml_dtypes.bfloat16))
    bvb = np.ascontiguousarray(
        np.broadcast_to(np.asarray(bv, np.float32)[None, :], (128, D))
    )

    # ps[b, s] = pos . qp with masked tokens forced to -1e30 so their
    # softmax weight underflows to exactly 0. Packed as [128 p, B, 4 t]
    # matching the device's s = 4p + t token scramble.
    ps = np.asarray(positions, np.float32) @ qp              # [B, S]
    m = np.asarray(mask, bool)
    if not m.all():
        ps = np.where(m, ps, np.float32(-1e30))
    ps = np.ascontiguousarray(
        ps.reshape(B, 128, TPB).transpose(1, 0, 2), np.float32
    )

    x = np.asarray(input_features, np.float32)
    in_maps = []
    for c in range(NCORES):
        in_maps.append(
            {
                "x": x[c * BSH : (c + 1) * BSH],
                "ps": np.ascontiguousarray(ps[:, c * BSH : (c + 1) * BSH]),
                "qkr": qkr,
                "wvt": wvt,
                "bvb": bvb,
            }
        )
    return in_maps


def kernel(input_features, positions, mask, query, Wk, bk, Wv, bv, Wp, bp):
    from concourse.bass_utils import run_bass_kernel_spmd

    if "nc" not in _CACHE:
        _CACHE["nc"] = build_program()
    nc = _CACHE["nc"]
    in_maps = prepare_inputs(
        input_features, positions, mask, query, Wk, bk, Wv, bv, Wp, bp
    )
    res = run_bass_kernel_spmd(nc, in_maps, list(range(NCORES)))
    return np.concatenate([res.results[c]["out"] for c in range(NCORES)], axis=0)


# revision 6
# speedup vs baseline: 2.0464x; 1.1621x over previous
"""AttentionPooling Trainium2 kernel (8-core data-parallel), v3.

Math per batch row b (B=2048, S=512, D=128):
    score[b,s] = x[b,s,:] . qk + ps[b,s]        (qk = Wk^T q * SCALE)
    out[b]     = (sum_s e_s x_s) @ Wv^T / (sum_s e_s) + bv,   e = exp(score)
ps = pos . qp (+ mask fold to -1e30) is tiny and precomputed host-side
(same O(B*S*4) folding the v1 kernel did).

Device layout (per core, BSH=256 batches): token scramble s = 4p + t, so
partition p holds 4 consecutive tokens of a batch and the x DMA moves
2 KB contiguous DRAM runs (16 queues hit line rate ~360 GB/s; the DMA
floor ~188 us/core is the roofline for this memory-regime problem).

Engine split per 8-batch granule [128p, 8b, 4t, 128d], balanced so DVE /
ACT / DMA all land ~185 us (measured per-op rates via microbench):
  DMA   x granule f32 in (2 KB packets)
  ACT   f32 -> bf16 convert (28 of 32 granules, ~4.6 us each) + exp
  GpSimd  the other 4 converts (~14 us each) + ps-add + bias adds
  DVE   scores: bf16 TT-mult against broadcast qk (no STT accumulator —
        its ~460 ns/instr fixed cost killed v2), 4 strided bf16 halving
        folds 128->8, segmented tensor_reduce 8->1 into f32 scores;
        plus L-partial reduce over t.
  PE    4 bf16 matmuls per batch (stationary x tile, moving e column)
        accumulating T[:,b] PSUM columns; per-block: ones-matmul for L,
        bf16 Wv^T projection, ACT 1/L per-partition scale, bias, store.
"""

import numpy as np

TOKEN_DIM = 128
SCALE = TOKEN_DIM ** -0.5
B, S, D = 2048, 512, 128
NCORES = 8
BSH = B // NCORES          # 256 batches per core
TPB = 4                    # tokens per partition per batch (s = 4p + t)
GR = 8                     # batches per granule
NGR = BSH // GR            # 32 granules per core
BLK = 128                  # batches per output block
GPB = BLK // GR            # granules per block (16)
NBLK = BSH // BLK          # 2
SEG = GR * TPB             # 32 score segments per granule
STG = 4                    # granules per exp/L staging group
GSB = STG * GR             # batches per staging group (32)

_CACHE = {}


def _split_multi_waits(nc):
    """The walrus build here rejects instructions carrying more than one
    semaphore wait (limit varies by ISA struct; STT and Drain allow 1).
    Hoist extra waits onto same-engine NoOps placed just before the
    instruction — identical blocking semantics, trivial cost."""
    from concourse import mybir

    n = 0
    for f in nc.m.functions:
        for bb in f.blocks:
            new = []
            for inst in bb.instructions:
                si = inst.sync_info
                if si is not None and si.on_wait and len(si.on_wait) > 1:
                    waits = list(si.on_wait)
                    for w in waits[1:]:
                        n += 1
                        nop = mybir.InstNoOp(
                            name=f"T-wsplit-{n}", engine=inst.engine, ins=[], outs=[]
                        )
                        nop.sync_info = mybir.SyncInfo(on_wait=[w], on_update=[])
                        new.append(nop)
                    inst.sync_info = mybir.SyncInfo(
                        on_wait=[waits[0]], on_update=list(si.on_update or [])
                    )
                new.append(inst)
            bb.instructions = new
    return n


def build_program():
    """Build the per-core Bass program (SPMD across the 8 cores)."""
    import concourse.bass as bass
    import concourse.tile as tile
    from concourse import mybir

    f32 = mybir.dt.float32
    bf16 = mybir.dt.bfloat16
    Exp = mybir.ActivationFunctionType.Exp
    Copy = mybir.ActivationFunctionType.Copy
    Add = mybir.AluOpType.add
    Mult = mybir.AluOpType.mult
    X = mybir.AxisListType.X

    nc = bass.Bass("TRN2", target_bir_lowering=False, debug=False)
    x_d = nc.dram_tensor("x", [BSH, S, D], f32, kind="ExternalInput").ap()
    ps_d = nc.dram_tensor("ps", [128, BSH, TPB], f32, kind="ExternalInput").ap()
    qkr_d = nc.dram_tensor("qkr", [128, TPB * D], bf16, kind="ExternalInput").ap()
    wvt_d = nc.dram_tensor("wvt", [D, D], bf16, kind="ExternalInput").ap()
    bvb_d = nc.dram_tensor("bvb", [128, D], f32, kind="ExternalInput").ap()
    out_d = nc.dram_tensor("out", [BSH, D], f32, kind="ExternalOutput").ap()

    with tile.TileContext(nc) as tc:
        with (
            tc.tile_pool(name="consts", bufs=1) as consts,
            tc.tile_pool(name="xf", bufs=5) as xf_pool,
            tc.tile_pool(name="xb", bufs=6) as xb_pool,
            tc.tile_pool(name="prod", bufs=2) as prod_pool,
            tc.tile_pool(name="f1", bufs=2) as f1_pool,
            tc.tile_pool(name="f2", bufs=2) as f2_pool,
            tc.tile_pool(name="f3", bufs=2) as f3_pool,
            tc.tile_pool(name="f4", bufs=2) as f4_pool,
            tc.tile_pool(name="sc", bufs=2) as sc_pool,
            tc.tile_pool(name="sce", bufs=2) as sce_pool,
            tc.tile_pool(name="e", bufs=2) as e_pool,
            tc.tile_pool(name="P", bufs=2) as P_pool,
            tc.tile_pool(name="tpsum", bufs=2, space="PSUM") as tpsum_pool,
            tc.tile_pool(name="epi_psum", bufs=2, space="PSUM") as epi_psum,
            tc.tile_pool(name="epi", bufs=4) as epi_pool,
        ):
            qkr_sb = consts.tile([128, TPB * D], bf16)
            nc.sync.dma_start(qkr_sb[:], qkr_d[:])
            wvt_sb = consts.tile([D, D], bf16)
            nc.sync.dma_start(wvt_sb[:], wvt_d[:])
            bvb_sb = consts.tile([128, D], f32)
            nc.sync.dma_start(bvb_sb[:], bvb_d[:])
            ps_sb = consts.tile([128, BSH, TPB], f32)
            nc.sync.dma_start(ps_sb[:], ps_d[:])
            ones_sb = consts.tile([128, 1], f32)
            nc.vector.memset(ones_sb[:], 1.0)

            qk_b = qkr_sb[:].rearrange("p (o f) -> p o f", o=1).broadcast_to(
                [128, GR, TPB * D]
            )

            for blk in range(NBLK):
                Tpsum = tpsum_pool.tile([128, BLK], f32)
                P_blk = P_pool.tile([128, BLK], f32)
                for sg in range(GPB // STG):
                    sc = sc_pool.tile([128, GSB, TPB], f32)
                    xbs = []
                    for si in range(STG):
                        gg = sg * STG + si
                        g = blk * GPB + gg
                        b0 = g * GR
                        xf = xf_pool.tile([128, GR, TPB, D], f32)
                        nc.sync.dma_start(
                            xf[:],
                            x_d[b0 : b0 + GR].rearrange(
                                "b (p t) d -> p b t d", t=TPB
                            ),
                        )
                        xb = xb_pool.tile([128, GR, TPB, D], bf16)
                        nc.scalar.activation(xb[:], xf[:], Copy)
                        xbs.append(xb)

                        # scores: prod = xb*qk (bf16), fold 128 -> 8, reduce
                        prod = prod_pool.tile([128, SEG, D], bf16)
                        nc.vector.tensor_tensor(
                            out=prod[:].rearrange("p s d -> p (s d)").rearrange(
                                "p (b f) -> p b f", b=GR
                            ),
                            in0=xb[:].rearrange("p b t d -> p b (t d)"),
                            in1=qk_b,
                            op=Mult,
                        )
                        f1 = f1_pool.tile([128, SEG, 64], bf16)
                        nc.vector.tensor_tensor(
                            out=f1[:], in0=prod[:, :, 0:64], in1=prod[:, :, 64:128],
                            op=Add,
                        )
                        f2 = f2_pool.tile([128, SEG, 32], bf16)
                        nc.vector.tensor_tensor(
                            out=f2[:], in0=f1[:, :, 0:32], in1=f1[:, :, 32:64], op=Add
                        )
                        f3 = f3_pool.tile([128, SEG, 16], bf16)
                        nc.vector.tensor_tensor(
                            out=f3[:], in0=f2[:, :, 0:16], in1=f2[:, :, 16:32], op=Add
                        )
                        f4 = f4_pool.tile([128, SEG, 8], bf16)
                        nc.vector.tensor_tensor(
                            out=f4[:], in0=f3[:, :, 0:8], in1=f3[:, :, 8:16], op=Add
                        )
                        nc.vector.tensor_reduce(
                            out=sc[:, si * GR : (si + 1) * GR, :].rearrange(
                                "p b t -> p (b t)"
                            ),
                            in_=f4[:], axis=X, op=Add,
                        )
                    # staged softmax prep over 32 batches
                    sb0 = (blk * GPB + sg * STG) * GR
                    sce = sce_pool.tile([128, GSB, TPB], f32)
                    nc.vector.tensor_tensor(
                        out=sce[:], in0=sc[:], in1=ps_sb[:, sb0 : sb0 + GSB, :], op=Add
                    )
                    e = e_pool.tile([128, GSB, TPB], bf16)
                    nc.scalar.activation(e[:], sce[:], Exp)
                    nc.vector.tensor_reduce(
                        out=P_blk[:, sg * GSB : (sg + 1) * GSB], in_=e[:],
                        axis=X, op=Add,
                    )
                    for si in range(STG):
                        for j in range(GR):
                            bcol = (sg * STG + si) * GR + j
                            for t in range(TPB):
                                nc.tensor.matmul(
                                    out=Tpsum[:, bcol : bcol + 1],
                                    lhsT=xbs[si][:, j, t, :],
                                    rhs=e[:, si * GR + j, t : t + 1],
                                    start=(t == 0),
                                    stop=(t == TPB - 1),
                                )
                # block epilogue
                Lp = epi_psum.tile([128, 1], f32, tag="Lp")
                nc.tensor.matmul(
                    out=Lp[:], lhsT=P_blk[:], rhs=ones_sb[:], start=True, stop=True
                )
                rcpL = epi_pool.tile([128, 1], f32, tag="rcpL")
                nc.vector.reciprocal(rcpL[:], Lp[:])
                Tsb = epi_pool.tile([128, BLK], bf16, tag="Tsb")
                nc.scalar.activation(Tsb[:], Tpsum[:], Copy)
                proj = epi_psum.tile([128, D], f32, tag="proj")
                nc.tensor.matmul(
                    out=proj[:], lhsT=Tsb[:], rhs=wvt_sb[:], start=True, stop=True
                )
                scaled = epi_pool.tile([128, D], f32, tag="scaled")
                nc.scalar.activation(scaled[:], proj[:], Copy, scale=rcpL[:])
                out_sb = epi_pool.tile([128, D], f32, tag="out_sb")
                nc.gpsimd.tensor_tensor(
                    out=out_sb[:], in0=scaled[:], in1=bvb_sb[:], op=Add
                )
                nc.sync.dma_start(out_d[blk * BLK : (blk + 1) * BLK, :], out_sb[:])

    _split_multi_waits(nc)
    return nc


def prepare_inputs(input_features, positions, mask, query, Wk, bk, Wv, bv, Wp, bp):
    """Host-side prep: shard along batch, replicate/fold the small weights."""
    import ml_dtypes

    q = np.asarray(query, np.float32)[0]
    qk = (q @ np.asarray(Wk, np.float32)) * SCALE           # [D]
    qp = (q @ np.asarray(Wp, np.float32)) * SCALE           # [4]
    qkr = np.ascontiguousarray(
        np.broadcast_to(
            np.tile(qk, TPB)[None, :].astype(ml_dtypes.bfloat16), (128, TPB * D)
        )
    )
    wvt = np.ascontiguousarray(np.asarray(Wv, np.float32).T.astype(ml_dtypes.bfloat16))
    bvb = np.ascontiguousarray(
        np.broadcast_to(np.asarray(bv, np.float32)[None, :], (128, D))
    )

    # ps[b, s] = pos . qp with masked tokens forced to -1e30 so their
    # softmax weight underflows to exactly 0. Packed as [128 p, B, 4 t]
    # matching the device's s = 4p + t token scramble.
    ps = np.asarray(positions, np.float32) @ qp              # [B, S]
    m = np.asarray(mask, bool)
    if not m.all():
        ps = np.where(m, ps, np.float32(-1e30))
    ps = np.ascontiguousarray(
        ps.reshape(B, 128, TPB).transpose(1, 0, 2), np.float32
    )

    x = np.asarray(input_features, np.float32)
    in_maps = []
    for c in range(NCORES):
        in_maps.append(
            {
                "x": x[c * BSH : (c + 1) * BSH],
                "ps": np.ascontiguousarray(ps[:, c * BSH : (c + 1) * BSH]),
                "qkr": qkr,
                "wvt": wvt,
                "bvb": bvb,
            }
        )
    return in_maps


def kernel(input_features, positions, mask, query, Wk, bk, Wv, bv, Wp, bp):
    from concourse.bass_utils import run_bass_kernel_spmd

    if "nc" not in _CACHE:
        _CACHE["nc"] = build_program()
    nc = _CACHE["nc"]
    in_maps = prepare_inputs(
        input_features, positions, mask, query, Wk, bk, Wv, bv, Wp, bp
    )
    res = run_bass_kernel_spmd(nc, in_maps, list(range(NCORES)))
    return np.concatenate([res.results[c]["out"] for c in range(NCORES)], axis=0)
